# revision 13
# baseline (speedup 1.0000x reference)
"""Trainium2 Bass kernel: 2-layer LSTM language-model loss.

Reference: x = embedding[features]; 2-layer LSTM over T=64 steps with
sequence-length state freezing; logits = out @ softmax_w + softmax_b;
masked mean cross-entropy -> scalar.

Strategy (8 NeuronCores, SPMD, zero cross-core collectives):
  * Every core runs the identical full-batch (B=128) recurrence; the
    large projection is sharded over the vocab dim (1250 cols/core).
    Host gathers: xent = log(sum_c S_c) - LD.
  * Recurrent/projection matmuls run in fp8e4 DoubleRow (2 k-chunks per
    instruction, 2x PE throughput); operands are 16-scaled so fp8 stays
    normal. Gate PSUM is 256x the true pre-activation; the tanh ACT
    reads with scale 0.5/256 and the cg gate's weight columns are
    pre-doubled so one scale serves all four gates (one ACT per bank).
  * Layer0's x-part gates are a host-precomputed table emb16 @ W0x16
    [V, 2048] bf16, gathered per step and preloaded into PSUM with 4
    identity matmuls (cheaper than 8 DoubleRow matmuls + transposes).
  * LD (= logit[label]) is a gathered softmax_w.T row dotted with o1 on
    DVE with accumulate - no per-chunk is_eq masks; computed identically
    on every core, host reads core 0's copy.
  * Cell elementwise is bf16 end-to-end; h carried 16-scaled. h
    transposes ride the DMA xbar (dma_start_transpose) with the fp8
    cast on the scalar engine; o transposes stay on the PE (critical
    path into the next matmul).
  * Emission interleaves cell1(t) with cell0(t+1) op-by-op: the two
    recurrence chains are data-independent, so each engine alternates
    between them instead of stalling on one chain's latency.

Assumes b0 = b1 = softmax_b = 0 (verified at runtime).
"""

import numpy as np
import ml_dtypes


def _ensure_path():
    try:
        import concourse  # noqa: F401
    except ImportError:
        import sys

        for p in ("/opt/trn_rl_repo", "/root/.axon_site/_ro/trn_rl_repo"):
            if p not in sys.path:
                sys.path.append(p)


_ensure_path()

from contextlib import ExitStack  # noqa: E402

import concourse.bass as bass  # noqa: E402
import concourse.bacc as bacc  # noqa: E402
import concourse.tile as tile  # noqa: E402
from concourse import mybir  # noqa: E402
from concourse.alu_op_type import AluOpType as OP  # noqa: E402
from concourse.bass import IndirectOffsetOnAxis  # noqa: E402
from concourse.bass_utils import run_bass_kernel_spmd  # noqa: E402
from concourse.masks import make_identity  # noqa: E402

dt = mybir.dt
AF = mybir.ActivationFunctionType
DR = mybir.MatmulPerfMode.DoubleRow

import os as _os

B = 128
T = int(_os.environ.get("KERNEL_T_OVERRIDE", "64"))
H = 512
V = 10000
NCORES = 8
VSH = V // NCORES  # 1250
G = 4 * H  # 2048
KH = H // 128  # 4 k-chunks per 512-wide contraction
FP8 = dt.float8e4
BF = dt.bfloat16
NP_FP8 = ml_dtypes.float8_e4m3
NP_BF = ml_dtypes.bfloat16
FSCALE = 16.0  # fp8 operand scale; products are 256x
PCHUNKS = [(0, 512), (512, 1024), (1024, VSH)]

_CACHE: dict = {}


def _emit(nc, tc, ext):
    f32 = dt.float32
    with ExitStack() as ctx:
        cpool = ctx.enter_context(tc.tile_pool(name="const", bufs=1))
        state = ctx.enter_context(tc.tile_pool(name="state", bufs=2))
        wp = ctx.enter_context(tc.tile_pool(name="work", bufs=4))
        gpsum = ctx.enter_context(tc.tile_pool(name="gpsum", bufs=2, space="PSUM"))
        tpsum = ctx.enter_context(tc.tile_pool(name="tpsum", bufs=1, space="PSUM"))
        ppsum = ctx.enter_context(tc.tile_pool(name="ppsum", bufs=1, space="PSUM"))

        # ---- constants / inputs -------------------------------------------
        feat = cpool.tile([B, T], dt.int32)
        nc.sync.dma_start(feat[:], ext["features"][:, :])
        lab_i = cpool.tile([B, T], dt.int32)
        nc.sync.dma_start(lab_i[:], ext["labels_i"][:, :])
        slen = cpool.tile([B, 1], f32)
        nc.sync.dma_start(slen[:], ext["seqlen"][:, :])

        w0h = cpool.tile([128, KH, G], FP8)
        for k in range(KH):
            nc.sync.dma_start(w0h[:, k, :], ext["w0h"][k, :, :])
        w1 = cpool.tile([128, 2 * KH, G], FP8)
        for k in range(2 * KH):
            nc.sync.dma_start(w1[:, k, :], ext["w1"][k, :, :])
        wsm = cpool.tile([128, KH, VSH], FP8)
        nc.sync.dma_start(wsm[:], ext["wsm"][:, :, :].rearrange("k p n -> p k n"))

        identb = cpool.tile([128, 128], BF)
        make_identity(nc, identb[:])

        iota_t = cpool.tile([128, T], f32)
        nc.gpsimd.iota(iota_t[:], pattern=[[1, T]], base=0, channel_multiplier=0,
                       allow_small_or_imprecise_dtypes=True)

        # masks: M = (t < seqlen), and derived per-step scalars
        M = cpool.tile([B, T], f32)
        nc.vector.tensor_scalar(out=M[:], in0=iota_t[:], scalar1=slen[:, 0:1],
                                scalar2=None, op0=OP.is_lt)
        Mh = cpool.tile([B, T], f32)  # 0.5*m
        nc.vector.tensor_scalar(out=Mh[:], in0=M[:], scalar1=0.5, scalar2=None,
                                op0=OP.mult)
        M1 = cpool.tile([B, T], f32)  # 1-m
        nc.vector.tensor_scalar(out=M1[:], in0=M[:], scalar1=-1.0, scalar2=1.0,
                                op0=OP.mult, op1=OP.add)
        M1h = cpool.tile([B, T], f32)  # 1-0.5*m
        nc.vector.tensor_scalar(out=M1h[:], in0=M[:], scalar1=-0.5, scalar2=1.0,
                                op0=OP.mult, op1=OP.add)
        M8 = cpool.tile([B, T], f32)  # 8*m  (osm carries the x16 h-scale)
        nc.vector.tensor_scalar(out=M8[:], in0=M[:], scalar1=8.0, scalar2=None,
                                op0=OP.mult)

        Sacc = cpool.tile([B, T], f32)
        LDacc = cpool.tile([B, T], f32)

        # ---- initial states ------------------------------------------------
        c_st = {}
        h_st = {}
        hT_st = {0: None, 1: None}
        for li in (0, 1):
            c_st[li] = state.tile([B, H], BF, name=f"c{li}", tag=f"c{li}")
            nc.vector.memset(c_st[li][:], 0.0)
            h_st[li] = state.tile([B, H], BF, name=f"h{li}", tag=f"h{li}")
            nc.vector.memset(h_st[li][:], 0.0)

        def k3(srcT):
            return srcT.rearrange("p (k b) -> p k b", k=KH)

        def alloc_gates():
            return [gpsum.tile([B, G // 2], f32, name="g", tag="g")
                    for _ in (0, 1)]

        def gates_part(halves, srcT, w_tile, k0, start, stop):
            # fp8 DoubleRow: one instruction covers a k-pair (256 of K)
            s3 = k3(srcT[:])
            for half in (0, 1):
                gh = halves[half]
                for n in (0, 1):
                    osl = slice(512 * n, 512 * (n + 1))
                    wsl = slice(1024 * half + 512 * n,
                                1024 * half + 512 * (n + 1))
                    for j in (0, 1):
                        nc.tensor.matmul(
                            gh[:, osl], s3[:, 2 * j:2 * j + 2, :],
                            w_tile[:, k0 + 2 * j:k0 + 2 * j + 2, wsl],
                            start=(start and j == 0),
                            stop=(stop and j == 1),
                            perf_mode=DR)

        def gather_xg(t):
            # layer0 x-part gates: one [B, 2048] bf16 row gather (256x scale)
            xg = wp.tile([B, G], BF, name="xg", tag="xg")
            nc.gpsimd.indirect_dma_start(
                out=xg[:], out_offset=None, in_=ext["xgtab"][:, :],
                in_offset=IndirectOffsetOnAxis(ap=feat[:, t:t + 1], axis=0))
            return xg

        def gather_wlab(t):
            wl = wp.tile([B, H], BF, name="wl", tag="wl")
            nc.gpsimd.indirect_dma_start(
                out=wl[:], out_offset=None, in_=ext["wsmT"][:, :],
                in_offset=IndirectOffsetOnAxis(ap=lab_i[:, t:t + 1], axis=0))
            return wl

        def preload_xg(halves, xg):
            # PSUM <- xg via identity matmuls (starts the accumulation group)
            for half in (0, 1):
                gh = halves[half]
                for n in (0, 1):
                    osl = slice(512 * n, 512 * (n + 1))
                    xsl = slice(1024 * half + 512 * n,
                                1024 * half + 512 * (n + 1))
                    nc.tensor.matmul(gh[:, osl], identb[:], xg[:, xsl],
                                     start=True, stop=False)

        def cell_gen(t, li, ghalves, make_hT, res):
            """bf16 LSTM cell, emitted as a generator so two independent
            cells can interleave op-by-op. Gate banks: A=[f,i], B=[o,cg]."""
            gA, gB = ghalves
            mht = Mh[:, t:t + 1]
            m1t = M1[:, t:t + 1]
            m1ht = M1h[:, t:t + 1]
            m8t = M8[:, t:t + 1]

            th = wp.tile([B, G], BF, name="th", tag=f"th{li}")
            nc.scalar.activation(th[:, 0:1024], gA[:], AF.Tanh, scale=0.5 / 256.0)
            yield
            fp = wp.tile([B, H], BF, name="fp", tag=f"fp{li}")
            nc.vector.tensor_scalar(out=fp[:], in0=th[:, 0:512], scalar1=mht,
                                    scalar2=m1ht, op0=OP.mult, op1=OP.add)
            nc.scalar.activation(th[:, 1024:2048], gB[:], AF.Tanh,
                                 scale=0.5 / 256.0)
            yield
            ip = wp.tile([B, H], BF, name="ip", tag=f"ip{li}")
            nc.vector.tensor_scalar(out=ip[:], in0=th[:, 512:1024], scalar1=mht,
                                    scalar2=mht, op0=OP.mult, op1=OP.add)
            yield
            r = wp.tile([B, H], BF, name="r", tag=f"r{li}")
            nc.vector.tensor_tensor(out=r[:], in0=fp[:], in1=c_st[li][:],
                                    op=OP.mult)
            yield
            q = wp.tile([B, H], BF, name="q", tag=f"q{li}")
            nc.vector.tensor_tensor(out=q[:], in0=ip[:], in1=th[:, 1536:2048],
                                    op=OP.mult)
            osm = wp.tile([B, H], BF, name="osm", tag=f"osm{li}")
            nc.vector.tensor_scalar(out=osm[:], in0=th[:, 1024:1536], scalar1=m8t,
                                    scalar2=m8t, op0=OP.mult, op1=OP.add)
            yield
            c_new = state.tile([B, H], BF, name=f"c{li}", tag=f"c{li}")
            nc.vector.tensor_tensor(out=c_new[:], in0=r[:], in1=q[:], op=OP.add)
            c_st[li] = c_new
            yield
            th_c = wp.tile([B, H], BF, name="th_c", tag=f"th_c{li}")
            nc.scalar.activation(th_c[:], c_new[:], AF.Tanh)
            yield
            o16 = wp.tile([B, H], BF, name=f"o{li}", tag=f"o{li}")
            nc.vector.tensor_tensor(out=o16[:], in0=osm[:], in1=th_c[:],
                                    op=OP.mult)
            res["o16"] = o16
            yield
            h16 = state.tile([B, H], BF, name=f"h{li}", tag=f"h{li}")
            nc.vector.scalar_tensor_tensor(out=h16[:], in0=h_st[li][:],
                                           scalar=m1t, in1=o16[:],
                                           op0=OP.mult, op1=OP.add)
            h_st[li] = h16
            yield
            # oT on the PE + DVE cast: feeds the next matmul directly
            ps = tpsum.tile([128, H], BF, name="tp", tag="tp")
            for kc in range(KH):
                sl = slice(128 * kc, 128 * (kc + 1))
                nc.tensor.transpose(ps[:, sl], o16[:, sl], identb[:])
            oT = wp.tile([128, H], FP8, name=f"oT{li}", tag=f"oT{li}")
            nc.vector.tensor_copy(out=oT[:], in_=ps[:])
            res["oT"] = oT
            yield
            if make_hT:
                ps2 = tpsum.tile([128, H], BF, name="tp", tag="tp")
                for kc in range(KH):
                    sl = slice(128 * kc, 128 * (kc + 1))
                    nc.tensor.transpose(ps2[:, sl], h16[:, sl], identb[:])
                hT = state.tile([128, H], FP8, name=f"hT{li}", tag=f"hT{li}")
                nc.vector.tensor_copy(out=hT[:], in_=ps2[:])
                hT_st[li] = hT

        def drive(*gens):
            live = [g for g in gens if g is not None]
            while live:
                for g in list(live):
                    try:
                        next(g)
                    except StopIteration:
                        live.remove(g)

        def project(t, o1T, o16_1, wl):
            s3 = k3(o1T[:])
            pp = ppsum.tile([128, 3 * 512], f32, name="pp", tag="pp")
            for (n0, n1) in PCHUNKS:
                w = n1 - n0
                for j in (0, 1):
                    nc.tensor.matmul(pp[:, n0:n0 + w], s3[:, 2 * j:2 * j + 2, :],
                                     wsm[:, 2 * j:2 * j + 2, n0:n1],
                                     start=(j == 0), stop=(j == 1),
                                     perf_mode=DR)
            # LD = o1 . softmax_w[:, label] via the gathered row (x256 scale)
            ld_scr = wp.tile([B, H], BF, name="ld_scr", tag="ld_scr")
            nc.vector.scalar_tensor_tensor(
                out=ld_scr[:], in0=wl[:], scalar=1.0, in1=o16_1[:],
                op0=OP.mult, op1=OP.mult,
                accum_out=LDacc[:, t:t + 1])
            exp_scr = wp.tile([B, VSH], f32, name="exp_scr", tag="exp_scr")
            nc.scalar.activation(exp_scr[:], pp[:, 0:VSH], AF.Exp,
                                 scale=1.0 / 256.0,
                                 accum_out=Sacc[:, t:t + 1])

        # ---- software-pipelined main loop ---------------------------------
        # Iteration t: cell0(t) is complete. Emit l1 gates (t), l0 gates
        # (t+1), projection (t-1), then drive cell1(t) and cell0(t+1)
        # interleaved (independent chains).
        xg0 = gather_xg(0)
        wl_cur = gather_wlab(0)
        g0 = alloc_gates()
        preload_xg(g0, xg0)
        xg_next = gather_xg(1) if T > 1 else None
        res0 = {}
        drive(cell_gen(0, 0, g0, make_hT=(T > 1), res=res0))
        prev1 = None  # (o1T, o16_1, wl) for project(t-1)
        for t in range(T):
            # projection + x-preload first: PE work that depends only on
            # last step's results, covering the oT/hT cast latency
            if prev1 is not None:
                project(t - 1, *prev1)
            res0_next = {}
            if t + 1 < T:
                g0 = alloc_gates()
                preload_xg(g0, xg_next)
                if t + 2 < T:
                    xg_next = gather_xg(t + 2)
            g1 = alloc_gates()
            if t > 0:
                gates_part(g1, hT_st[1], w1, KH, start=True, stop=False)
            gates_part(g1, res0["oT"], w1, 0, start=(t == 0), stop=True)
            if t + 1 < T:
                gates_part(g0, hT_st[0], w0h, 0, start=False, stop=True)
            wl_next = gather_wlab(t + 1) if t + 1 < T else None
            res1 = {}
            drive(cell_gen(t, 1, g1, make_hT=(t + 1 < T), res=res1),
                  cell_gen(t + 1, 0, g0, make_hT=(t + 2 < T), res=res0_next)
                  if t + 1 < T else None)
            prev1 = (res1["oT"], res1["o16"], wl_cur)
            wl_cur = wl_next
            res0 = res0_next
        project(T - 1, *prev1)

        nc.sync.dma_start(ext["S"][:, :], Sacc[:])
        nc.sync.dma_start(ext["LD"][:, :], LDacc[:])


def _build():
    if "nc" in _CACHE:
        return _CACHE["nc"]
    nc = bacc.Bacc("TRN2", target_bir_lowering=False, debug=False,
                   num_devices=NCORES)
    ext = {
        "features": nc.declare_dram_parameter("features", [B, T], dt.int32,
                                              isOutput=False),
        "labels_i": nc.declare_dram_parameter("labels_i", [B, T], dt.int32,
                                              isOutput=False),
        "seqlen": nc.declare_dram_parameter("seqlen", [B, 1], dt.float32,
                                            isOutput=False),
        "xgtab": nc.declare_dram_parameter("xgtab", [V, G], BF, isOutput=False),
        "wsmT": nc.declare_dram_parameter("wsmT", [V, H], BF, isOutput=False),
        "w0h": nc.declare_dram_parameter("w0h", [KH, 128, G], FP8,
                                         isOutput=False),
        "w1": nc.declare_dram_parameter("w1", [2 * KH, 128, G], FP8,
                                        isOutput=False),
        "wsm": nc.declare_dram_parameter("wsm", [KH, 128, VSH], FP8,
                                         isOutput=False),
        "S": nc.declare_dram_parameter("S", [B, T], dt.float32, isOutput=True),
        "LD": nc.declare_dram_parameter("LD", [B, T], dt.float32,
                                        isOutput=True),
    }
    with tile.TileContext(nc) as tc:
        _emit(nc, tc, ext)
    nc.compile()
    _CACHE["nc"] = nc
    return nc


def _reorder_cols(w):
    # gate blocks [i, cg, f, o] -> [f, i, o, cg], cg doubled (one tanh scale)
    return np.concatenate([w[:, 1024:1536], w[:, 0:512], w[:, 1536:2048],
                           2.0 * w[:, 512:1024]], axis=1)


def _pack_wh(Wh):
    w = _reorder_cols(np.asarray(Wh, np.float32)) * np.float32(FSCALE)
    return np.ascontiguousarray(w.reshape(KH, 128, G)).astype(NP_FP8)


def _pack_w1(W1x, W1h):
    w = np.concatenate([np.asarray(W1x, np.float32),
                        np.asarray(W1h, np.float32)], axis=0)
    w = _reorder_cols(w) * np.float32(FSCALE)
    return np.ascontiguousarray(w.reshape(2 * KH, 128, G)).astype(NP_FP8)


def kernel(features, labels, seq_lengths, seq_mask, embedding,
           W0x, W0h, b0, W1x, W1h, b1, softmax_w, softmax_b,
           _trace_dir=None):
    for name, b in (("b0", b0), ("b1", b1), ("softmax_b", softmax_b)):
        if np.any(np.asarray(b, np.float32) != 0.0):
            raise NotImplementedError(f"{name} != 0 not supported")

    feats = np.ascontiguousarray(np.asarray(features, np.int32)[:, :T])
    labels_i = np.ascontiguousarray(np.asarray(labels, np.int32)[:, :T])
    slen = np.asarray(seq_lengths, np.int32).astype(np.float32).reshape(B, 1)
    mask = np.asarray(seq_mask, np.float32)[:, :T]

    emb32 = np.asarray(embedding, np.float32)
    # layer0 x-part table: 256x-scaled gates in the kernel's column order
    xgtab = (emb32 @ _reorder_cols(np.asarray(W0x, np.float32))
             * np.float32(FSCALE * FSCALE)).astype(NP_BF)
    wsmT = np.ascontiguousarray(
        (np.asarray(softmax_w, np.float32) * np.float32(FSCALE)).T).astype(NP_BF)
    w0h = _pack_wh(W0h)
    w1 = _pack_w1(W1x, W1h)
    wsm_r = (np.asarray(softmax_w, np.float32) * np.float32(FSCALE)).reshape(
        KH, 128, V)

    nc = _build()
    in_maps = []
    for c in range(NCORES):
        in_maps.append({
            "features": feats,
            "labels_i": labels_i,
            "seqlen": slen,
            "xgtab": xgtab,
            "wsmT": wsmT,
            "w0h": w0h,
            "w1": w1,
            "wsm": np.ascontiguousarray(
                wsm_r[:, :, c * VSH:(c + 1) * VSH]).astype(NP_FP8),
        })

    kwargs = {}
    if _trace_dir is not None:
        kwargs = dict(trace=True, tmpdir=_trace_dir)
    res = run_bass_kernel_spmd(nc, in_maps, list(range(NCORES)), **kwargs)
    _CACHE["last_results"] = res

    S = np.zeros((B, T), np.float64)
    for c in range(NCORES):
        S += np.asarray(res.results[c]["S"], np.float64)
    LD = np.asarray(res.results[0]["LD"], np.float64) / 256.0

    xent = np.log(S) - LD
    loss_t = (xent * mask).sum(axis=0) / (mask.sum(axis=0) + 1e-12)
    cost = loss_t.mean()
    return np.asarray(cost, np.float32)


# revision 14
# speedup vs baseline: 1.1150x; 1.1150x over previous
"""Trainium2 Bass kernel: 2-layer LSTM language-model loss.

Reference: x = embedding[features]; 2-layer LSTM over T=64 steps with
sequence-length state freezing; logits = out @ softmax_w + softmax_b;
masked mean cross-entropy -> scalar.

Strategy (8 NeuronCores, SPMD, zero cross-core collectives):
  * Every core runs the identical full-batch (B=128) recurrence; the
    large projection is sharded over the vocab dim (1250 cols/core).
    Host gathers: xent = log(sum_c S_c) - sum_c LD_c.
  * All big matmuls run in fp8e4 with DoubleRow perf mode (2 k-chunks
    of 128 per instruction): weights, embedding and the h/o activations
    are scaled by 16 host/kernel-side so fp8 values stay normal; gate
    PSUM is 256x the true pre-activation and the tanh ACT reads with
    scale 0.5/256 (the cg gate's weight columns are pre-doubled so one
    scale serves all four gates -> one ACT per PSUM bank).
  * Cell elementwise chain is bf16 end-to-end (DVE 2x/4x modes); c and
    h are bf16 (h carried 16-scaled so transposed copies cast straight
    to fp8 without a multiply).
  * Masking is folded into the sigmoid affines as in the baseline:
    sig(x) = 0.5 tanh(0.5 x) + 0.5, fp = 0.5m th + (1 - 0.5m) etc.
  * S and LD are emitted per projection chunk ([B, 3T]) so the kernel
    skips the per-step add trees; the host sums the 3 chunks.

Assumes b0 = b1 = softmax_b = 0 (verified at runtime).
"""

import numpy as np
import ml_dtypes


def _ensure_path():
    try:
        import concourse  # noqa: F401
    except ImportError:
        import sys

        for p in ("/opt/trn_rl_repo", "/root/.axon_site/_ro/trn_rl_repo"):
            if p not in sys.path:
                sys.path.append(p)


_ensure_path()

from contextlib import ExitStack  # noqa: E402

import concourse.bass as bass  # noqa: E402
import concourse.bacc as bacc  # noqa: E402
import concourse.tile as tile  # noqa: E402
from concourse import mybir  # noqa: E402
from concourse.alu_op_type import AluOpType as OP  # noqa: E402
from concourse.bass import IndirectOffsetOnAxis  # noqa: E402
from concourse.bass_utils import run_bass_kernel_spmd  # noqa: E402
from concourse.masks import make_identity  # noqa: E402

dt = mybir.dt
AF = mybir.ActivationFunctionType
DR = mybir.MatmulPerfMode.DoubleRow

import os as _os

B = 128
T = int(_os.environ.get("KERNEL_T_OVERRIDE", "64"))
H = 512
V = 10000
NCORES = 8
VSH = V // NCORES  # 1250
G = 4 * H  # 2048
KH = H // 128  # 4 k-chunks per 512-wide contraction
FP8 = dt.float8e4
BF = dt.bfloat16
NP_FP8 = ml_dtypes.float8_e4m3
FSCALE = 16.0  # fp8 operand scale; products are 256x
# projection free-dim chunks (PSUM bank = 512 fp32)
PCHUNKS = [(0, 512), (512, 1024), (1024, VSH)]
NPC = len(PCHUNKS)

_CACHE: dict = {}


def _emit(nc, tc, ext):
    f32 = dt.float32
    with ExitStack() as ctx:
        cpool = ctx.enter_context(tc.tile_pool(name="const", bufs=1))
        state = ctx.enter_context(tc.tile_pool(name="state", bufs=2))
        wp = ctx.enter_context(tc.tile_pool(name="work", bufs=3))
        gpsum = ctx.enter_context(tc.tile_pool(name="gpsum", bufs=2, space="PSUM"))
        tpsum = ctx.enter_context(tc.tile_pool(name="tpsum", bufs=2, space="PSUM"))
        ppsum = ctx.enter_context(tc.tile_pool(name="ppsum", bufs=2, space="PSUM"))

        # ---- constants / inputs -------------------------------------------
        feat = cpool.tile([B, T], dt.int32)
        nc.sync.dma_start(feat[:], ext["features"][:, :])
        lab = cpool.tile([B, T], f32)
        nc.sync.dma_start(lab[:], ext["labels_sh"][:, :])
        slen = cpool.tile([B, 1], f32)
        nc.sync.dma_start(slen[:], ext["seqlen"][:, :])

        # per-k-chunk DMAs: first gate matmuls start before the full load
        w0 = cpool.tile([128, 2 * KH, G], FP8)
        for k in range(2 * KH):
            nc.sync.dma_start(w0[:, k, :], ext["w0"][k, :, :])
        w1 = cpool.tile([128, 2 * KH, G], FP8)
        for k in range(2 * KH):
            nc.sync.dma_start(w1[:, k, :], ext["w1"][k, :, :])
        wsm = cpool.tile([128, KH, VSH], FP8)
        nc.sync.dma_start(wsm[:], ext["wsm"][:, :, :].rearrange("k p n -> p k n"))

        identb = cpool.tile([128, 128], BF)
        make_identity(nc, identb[:])

        iota_v = cpool.tile([128, VSH], f32)
        nc.gpsimd.iota(iota_v[:], pattern=[[1, VSH]], base=0, channel_multiplier=0,
                       allow_small_or_imprecise_dtypes=True)
        iota_t = cpool.tile([128, T], f32)
        nc.gpsimd.iota(iota_t[:], pattern=[[1, T]], base=0, channel_multiplier=0,
                       allow_small_or_imprecise_dtypes=True)

        # masks: M = (t < seqlen), and derived per-step scalars
        M = cpool.tile([B, T], f32)
        nc.vector.tensor_scalar(out=M[:], in0=iota_t[:], scalar1=slen[:, 0:1],
                                scalar2=None, op0=OP.is_lt)
        Mh = cpool.tile([B, T], f32)  # 0.5*m
        nc.vector.tensor_scalar(out=Mh[:], in0=M[:], scalar1=0.5, scalar2=None,
                                op0=OP.mult)
        M1 = cpool.tile([B, T], f32)  # 1-m
        nc.vector.tensor_scalar(out=M1[:], in0=M[:], scalar1=-1.0, scalar2=1.0,
                                op0=OP.mult, op1=OP.add)
        M1h = cpool.tile([B, T], f32)  # 1-0.5*m
        nc.vector.tensor_scalar(out=M1h[:], in0=M[:], scalar1=-0.5, scalar2=1.0,
                                op0=OP.mult, op1=OP.add)
        M8 = cpool.tile([B, T], f32)  # 8*m  (osm carries the x16 h-scale)
        nc.vector.tensor_scalar(out=M8[:], in0=M[:], scalar1=8.0, scalar2=None,
                                op0=OP.mult)

        Sacc = cpool.tile([B, NPC * T], f32)
        LDacc = cpool.tile([B, NPC * T], f32)

        # ---- initial states ------------------------------------------------
        c_st = {}
        h_st = {}
        hT_st = {0: None, 1: None}
        for li in (0, 1):
            c_st[li] = state.tile([B, H], BF, name=f"c{li}", tag=f"c{li}")
            nc.vector.memset(c_st[li][:], 0.0)
            h_st[li] = state.tile([B, H], BF, name=f"h{li}", tag=f"h{li}")
            nc.vector.memset(h_st[li][:], 0.0)

        def k3(srcT):
            # [128, 512] fp8 T-layout -> [128, 4, 128] (k-chunk, batch)
            return srcT.rearrange("p (k b) -> p k b", k=KH)

        def alloc_gates():
            return [gpsum.tile([B, G // 2], f32, name="g", tag="g")
                    for _ in (0, 1)]

        def gates_part(halves, srcT, w_tile, part, start, stop):
            # fp8 DoubleRow: one instruction covers a k-pair (256 of K)
            k0 = 0 if part == "x" else KH
            s3 = k3(srcT[:])
            for half in (0, 1):
                gh = halves[half]
                for n in (0, 1):
                    osl = slice(512 * n, 512 * (n + 1))
                    wsl = slice(1024 * half + 512 * n,
                                1024 * half + 512 * (n + 1))
                    for j in (0, 1):
                        nc.tensor.matmul(
                            gh[:, osl], s3[:, 2 * j:2 * j + 2, :],
                            w_tile[:, k0 + 2 * j:k0 + 2 * j + 2, wsl],
                            start=(start and j == 0),
                            stop=(stop and j == 1),
                            perf_mode=DR)

        def cell(t, li, ghalves, make_hT):
            """bf16 LSTM cell. Gate banks: A=[f,i], B=[o,cg] (cg weights
            pre-doubled so one tanh scale serves all gates). h is carried
            16-scaled so the transposed copies cast straight to fp8."""
            gA, gB = ghalves
            mht = Mh[:, t:t + 1]
            m1t = M1[:, t:t + 1]
            m1ht = M1h[:, t:t + 1]
            m8t = M8[:, t:t + 1]

            th = wp.tile([B, G], BF, name="th", tag="th")
            nc.scalar.activation(th[:, 0:1024], gA[:], AF.Tanh, scale=0.5 / 256.0)
            nc.scalar.activation(th[:, 1024:2048], gB[:], AF.Tanh,
                                 scale=0.5 / 256.0)

            fp = wp.tile([B, H], BF, name="fp", tag="fp")
            nc.vector.tensor_scalar(out=fp[:], in0=th[:, 0:512], scalar1=mht,
                                    scalar2=m1ht, op0=OP.mult, op1=OP.add)
            ip = wp.tile([B, H], BF, name="ip", tag="ip")
            nc.vector.tensor_scalar(out=ip[:], in0=th[:, 512:1024], scalar1=mht,
                                    scalar2=mht, op0=OP.mult, op1=OP.add)
            osm = wp.tile([B, H], BF, name="osm", tag="osm")
            nc.vector.tensor_scalar(out=osm[:], in0=th[:, 1024:1536], scalar1=m8t,
                                    scalar2=m8t, op0=OP.mult, op1=OP.add)
            q = wp.tile([B, H], BF, name="q", tag="q")
            nc.vector.tensor_tensor(out=q[:], in0=ip[:], in1=th[:, 1536:2048],
                                    op=OP.mult)
            r = wp.tile([B, H], BF, name="r", tag="r")
            nc.vector.tensor_tensor(out=r[:], in0=fp[:], in1=c_st[li][:],
                                    op=OP.mult)
            c_new = state.tile([B, H], BF, name=f"c{li}", tag=f"c{li}")
            nc.vector.tensor_tensor(out=c_new[:], in0=r[:], in1=q[:], op=OP.add)
            c_st[li] = c_new

            th_c = wp.tile([B, H], BF, name="th_c", tag="th_c")
            nc.scalar.activation(th_c[:], c_new[:], AF.Tanh)

            o16 = wp.tile([B, H], BF, name=f"o{li}", tag=f"o{li}")
            nc.vector.tensor_tensor(out=o16[:], in0=osm[:], in1=th_c[:],
                                    op=OP.mult)
            h16 = state.tile([B, H], BF, name=f"h{li}", tag=f"h{li}")
            nc.vector.scalar_tensor_tensor(out=h16[:], in0=h_st[li][:],
                                           scalar=m1t, in1=o16[:],
                                           op0=OP.mult, op1=OP.add)
            h_st[li] = h16

            ps = tpsum.tile([128, H], BF, name="tp", tag="tp")
            for kc in range(KH):
                sl = slice(128 * kc, 128 * (kc + 1))
                nc.tensor.transpose(ps[:, sl], o16[:, sl], identb[:])
            oT = wp.tile([128, H], FP8, name=f"oT{li}", tag=f"oT{li}")
            nc.vector.tensor_copy(out=oT[:], in_=ps[:])

            if make_hT:
                ps2 = tpsum.tile([128, H], BF, name="tp", tag="tp")
                for kc in range(KH):
                    sl = slice(128 * kc, 128 * (kc + 1))
                    nc.tensor.transpose(ps2[:, sl], h16[:, sl], identb[:])
                hT = state.tile([128, H], FP8, name=f"hT{li}", tag=f"hT{li}")
                nc.vector.tensor_copy(out=hT[:], in_=ps2[:])
                hT_st[li] = hT
            return oT

        def gather_x(t):
            xg = wp.tile([B, H], BF, name="xg", tag="xg")
            nc.gpsimd.indirect_dma_start(
                out=xg[:], out_offset=None, in_=ext["emb"][:, :],
                in_offset=IndirectOffsetOnAxis(ap=feat[:, t:t + 1], axis=0))
            ps = tpsum.tile([128, H], BF, name="tp", tag="tp")
            for kc in range(KH):
                sl = slice(128 * kc, 128 * (kc + 1))
                nc.tensor.transpose(ps[:, sl], xg[:, sl], identb[:])
            xT = wp.tile([128, H], FP8, name="xT", tag="xT")
            nc.vector.tensor_copy(out=xT[:], in_=ps[:])
            return xT

        def project(t, o1T):
            s3 = k3(o1T[:])
            for ci, (n0, n1) in enumerate(PCHUNKS):
                w = n1 - n0
                pp = ppsum.tile([128, 512], f32, name="pp", tag="pp")
                for j in (0, 1):
                    nc.tensor.matmul(pp[:, 0:w], s3[:, 2 * j:2 * j + 2, :],
                                     wsm[:, 2 * j:2 * j + 2, n0:n1],
                                     start=(j == 0), stop=(j == 1),
                                     perf_mode=DR)
                stt_scr = wp.tile([B, 512], f32, name="stt_scr", tag="stt_scr")
                nc.vector.scalar_tensor_tensor(
                    out=stt_scr[:, 0:w], in0=iota_v[:, n0:n1],
                    scalar=lab[:, t:t + 1], in1=pp[:, 0:w],
                    op0=OP.is_equal, op1=OP.mult,
                    accum_out=LDacc[:, NPC * t + ci:NPC * t + ci + 1])
                exp_scr = wp.tile([B, 512], f32, name="exp_scr", tag="exp_scr")
                nc.scalar.activation(
                    exp_scr[:, 0:w], pp[:, 0:w], AF.Exp, scale=1.0 / 256.0,
                    accum_out=Sacc[:, NPC * t + ci:NPC * t + ci + 1])

        # ---- software-pipelined main loop ---------------------------------
        xT_cur = gather_x(0)
        g0 = alloc_gates()
        gates_part(g0, xT_cur, w0, "x", start=True, stop=True)  # t=0: no rec
        o1T_prev = None
        for t in range(T):
            if t > 0:
                gates_part(g0, hT_st[0], w0, "h", start=False, stop=True)
            if t + 1 < T:
                xT_next = gather_x(t + 1)
            if o1T_prev is not None:
                project(t - 1, o1T_prev)
            g1 = None
            if t > 0:
                g1 = alloc_gates()
                gates_part(g1, hT_st[1], w1, "h", start=True, stop=False)
            o0T = cell(t, 0, g0, make_hT=(t + 1 < T))
            if g1 is None:
                g1 = alloc_gates()
                gates_part(g1, o0T, w1, "x", start=True, stop=True)
            else:
                gates_part(g1, o0T, w1, "x", start=False, stop=True)
            if t + 1 < T:
                g0 = alloc_gates()
                gates_part(g0, xT_next, w0, "x", start=True, stop=False)
                xT_cur = xT_next
            o1T = cell(t, 1, g1, make_hT=(t + 1 < T))
            o1T_prev = o1T
        project(T - 1, o1T_prev)

        nc.sync.dma_start(ext["S"][:, :], Sacc[:])
        nc.sync.dma_start(ext["LD"][:, :], LDacc[:])


def _build():
    if "nc" in _CACHE:
        return _CACHE["nc"]
    nc = bacc.Bacc("TRN2", target_bir_lowering=False, debug=False,
                   num_devices=NCORES)
    ext = {
        "features": nc.declare_dram_parameter("features", [B, T], dt.int32,
                                              isOutput=False),
        "labels_sh": nc.declare_dram_parameter("labels_sh", [B, T], dt.float32,
                                               isOutput=False),
        "seqlen": nc.declare_dram_parameter("seqlen", [B, 1], dt.float32,
                                            isOutput=False),
        "emb": nc.declare_dram_parameter("emb", [V, H], BF, isOutput=False),
        "w0": nc.declare_dram_parameter("w0", [2 * KH, 128, G], FP8,
                                        isOutput=False),
        "w1": nc.declare_dram_parameter("w1", [2 * KH, 128, G], FP8,
                                        isOutput=False),
        "wsm": nc.declare_dram_parameter("wsm", [KH, 128, VSH], FP8,
                                         isOutput=False),
        "S": nc.declare_dram_parameter("S", [B, NPC * T], dt.float32,
                                       isOutput=True),
        "LD": nc.declare_dram_parameter("LD", [B, NPC * T], dt.float32,
                                        isOutput=True),
    }
    with tile.TileContext(nc) as tc:
        _emit(nc, tc, ext)
    nc.compile()
    _CACHE["nc"] = nc
    return nc


def _pack_w(Wx, Wh):
    w = np.concatenate([np.asarray(Wx, np.float32), np.asarray(Wh, np.float32)],
                       axis=0)  # [2H, 4H] rows: x-part then h-part
    # reorder gate blocks [i, cg, f, o] -> [f, i, o, cg]
    w = np.concatenate([w[:, 1024:1536], w[:, 0:512], w[:, 1536:2048],
                        w[:, 512:1024]], axis=1)
    w = w * np.float32(FSCALE)
    w[:, 1536:2048] *= np.float32(2.0)  # cg: unified 0.5 tanh scale
    return np.ascontiguousarray(w.reshape(2 * KH, 128, G)).astype(NP_FP8)


def kernel(features, labels, seq_lengths, seq_mask, embedding,
           W0x, W0h, b0, W1x, W1h, b1, softmax_w, softmax_b,
           _trace_dir=None):
    for name, b in (("b0", b0), ("b1", b1), ("softmax_b", softmax_b)):
        if np.any(np.asarray(b, np.float32) != 0.0):
            raise NotImplementedError(f"{name} != 0 not supported")

    feats = np.ascontiguousarray(np.asarray(features, np.int32)[:, :T])
    labels_f = np.ascontiguousarray(
        np.asarray(labels, np.int32)[:, :T].astype(np.float32))
    slen = np.asarray(seq_lengths, np.int32).astype(np.float32).reshape(B, 1)
    mask = np.asarray(seq_mask, np.float32)[:, :T]
    emb = (np.asarray(embedding, np.float32) * np.float32(FSCALE)).astype(
        ml_dtypes.bfloat16)
    w0 = _pack_w(W0x, W0h)
    w1 = _pack_w(W1x, W1h)
    wsm_r = (np.asarray(softmax_w, np.float32) * np.float32(FSCALE)).reshape(
        KH, 128, V)

    nc = _build()
    in_maps = []
    for c in range(NCORES):
        in_maps.append({
            "features": feats,
            "labels_sh": labels_f - np.float32(c * VSH),
            "seqlen": slen,
            "emb": emb,
            "w0": w0,
            "w1": w1,
            "wsm": np.ascontiguousarray(
                wsm_r[:, :, c * VSH:(c + 1) * VSH]).astype(NP_FP8),
        })

    kwargs = {}
    if _trace_dir is not None:
        kwargs = dict(trace=True, tmpdir=_trace_dir)
    res = run_bass_kernel_spmd(nc, in_maps, list(range(NCORES)), **kwargs)
    _CACHE["last_results"] = res

    S = np.zeros((B, T), np.float64)
    LD = np.zeros((B, T), np.float64)
    for c in range(NCORES):
        S += np.asarray(res.results[c]["S"], np.float64).reshape(B, T, NPC).sum(-1)
        LD += np.asarray(res.results[c]["LD"], np.float64).reshape(B, T, NPC).sum(-1)
    LD /= 256.0

    xent = np.log(S) - LD
    loss_t = (xent * mask).sum(axis=0) / (mask.sum(axis=0) + 1e-12)
    cost = loss_t.mean()
    return np.asarray(cost, np.float32)


# revision 15
# speedup vs baseline: 1.1287x; 1.0123x over previous
"""Trainium2 Bass kernel: 2-layer LSTM language-model loss.

Reference: x = embedding[features]; 2-layer LSTM over T=64 steps with
sequence-length state freezing; logits = out @ softmax_w + softmax_b;
masked mean cross-entropy -> scalar.

Strategy (8 NeuronCores, SPMD, zero cross-core collectives):
  * Every core runs the identical full-batch (B=128) recurrence; the
    large projection is sharded over the vocab dim (1250 cols/core).
    Host gathers: xent = log(sum_c S_c) - sum_c LD_c.
  * All big matmuls run in fp8e4 with DoubleRow perf mode (2 k-chunks
    of 128 per instruction): weights, embedding and the h/o activations
    are scaled by 16 host/kernel-side so fp8 values stay normal; gate
    PSUM is 256x the true pre-activation and the tanh ACT reads with
    scale 0.5/256 (the cg gate's weight columns are pre-doubled so one
    scale serves all four gates -> one ACT per PSUM bank).
  * Cell elementwise chain is bf16 end-to-end (DVE 2x/4x modes); c and
    h are bf16 (h carried 16-scaled so transposed copies cast straight
    to fp8 without a multiply).
  * Masking is folded into the sigmoid affines as in the baseline:
    sig(x) = 0.5 tanh(0.5 x) + 0.5, fp = 0.5m th + (1 - 0.5m) etc.
  * S and LD are emitted per projection chunk ([B, 3T]) so the kernel
    skips the per-step add trees; the host sums the 3 chunks.

Assumes b0 = b1 = softmax_b = 0 (verified at runtime).
"""

import numpy as np
import ml_dtypes


def _ensure_path():
    try:
        import concourse  # noqa: F401
    except ImportError:
        import sys

        for p in ("/opt/trn_rl_repo", "/root/.axon_site/_ro/trn_rl_repo"):
            if p not in sys.path:
                sys.path.append(p)


_ensure_path()

from contextlib import ExitStack  # noqa: E402

import concourse.bass as bass  # noqa: E402
import concourse.bacc as bacc  # noqa: E402
import concourse.tile as tile  # noqa: E402
from concourse import mybir  # noqa: E402
from concourse.alu_op_type import AluOpType as OP  # noqa: E402
from concourse.bass import IndirectOffsetOnAxis  # noqa: E402
from concourse.bass_utils import run_bass_kernel_spmd  # noqa: E402
from concourse.masks import make_identity  # noqa: E402

dt = mybir.dt
AF = mybir.ActivationFunctionType
DR = mybir.MatmulPerfMode.DoubleRow

import os as _os

B = 128
T = int(_os.environ.get("KERNEL_T_OVERRIDE", "64"))
H = 512
V = 10000
NCORES = 8
VSH = V // NCORES  # 1250
G = 4 * H  # 2048
KH = H // 128  # 4 k-chunks per 512-wide contraction
FP8 = dt.float8e4
BF = dt.bfloat16
NP_FP8 = ml_dtypes.float8_e4m3
FSCALE = 16.0  # fp8 operand scale; products are 256x
# projection free-dim chunks (PSUM bank = 512 fp32)
PCHUNKS = [(0, 512), (512, 1024), (1024, VSH)]
NPC = len(PCHUNKS)

_CACHE: dict = {}


def _emit(nc, tc, ext):
    f32 = dt.float32
    with ExitStack() as ctx:
        cpool = ctx.enter_context(tc.tile_pool(name="const", bufs=1))
        state = ctx.enter_context(tc.tile_pool(name="state", bufs=2))
        wp = ctx.enter_context(tc.tile_pool(name="work", bufs=3))
        gpsum = ctx.enter_context(tc.tile_pool(name="gpsum", bufs=2, space="PSUM"))
        tpsum = ctx.enter_context(tc.tile_pool(name="tpsum", bufs=2, space="PSUM"))
        ppsum = ctx.enter_context(tc.tile_pool(name="ppsum", bufs=2, space="PSUM"))

        # ---- constants / inputs -------------------------------------------
        feat = cpool.tile([B, T], dt.int32)
        nc.sync.dma_start(feat[:], ext["features"][:, :])
        lab = cpool.tile([B, T], f32)
        nc.sync.dma_start(lab[:], ext["labels_sh"][:, :])
        slen = cpool.tile([B, 1], f32)
        nc.sync.dma_start(slen[:], ext["seqlen"][:, :])

        # per-k-chunk DMAs: first gate matmuls start before the full load
        w0 = cpool.tile([128, 2 * KH, G], FP8)
        for k in range(2 * KH):
            nc.sync.dma_start(w0[:, k, :], ext["w0"][k, :, :])
        w1 = cpool.tile([128, 2 * KH, G], FP8)
        for k in range(2 * KH):
            nc.sync.dma_start(w1[:, k, :], ext["w1"][k, :, :])
        wsm = cpool.tile([128, KH, VSH], FP8)
        nc.sync.dma_start(wsm[:], ext["wsm"][:, :, :].rearrange("k p n -> p k n"))

        identb = cpool.tile([128, 128], BF)
        make_identity(nc, identb[:])

        iota_v = cpool.tile([128, VSH], f32)
        nc.gpsimd.iota(iota_v[:], pattern=[[1, VSH]], base=0, channel_multiplier=0,
                       allow_small_or_imprecise_dtypes=True)
        iota_t = cpool.tile([128, T], f32)
        nc.gpsimd.iota(iota_t[:], pattern=[[1, T]], base=0, channel_multiplier=0,
                       allow_small_or_imprecise_dtypes=True)

        # masks: M = (t < seqlen), and derived per-step scalars
        M = cpool.tile([B, T], f32)
        nc.vector.tensor_scalar(out=M[:], in0=iota_t[:], scalar1=slen[:, 0:1],
                                scalar2=None, op0=OP.is_lt)
        Mh = cpool.tile([B, T], f32)  # 0.5*m
        nc.vector.tensor_scalar(out=Mh[:], in0=M[:], scalar1=0.5, scalar2=None,
                                op0=OP.mult)
        M1 = cpool.tile([B, T], f32)  # 1-m
        nc.vector.tensor_scalar(out=M1[:], in0=M[:], scalar1=-1.0, scalar2=1.0,
                                op0=OP.mult, op1=OP.add)
        M1h = cpool.tile([B, T], f32)  # 1-0.5*m
        nc.vector.tensor_scalar(out=M1h[:], in0=M[:], scalar1=-0.5, scalar2=1.0,
                                op0=OP.mult, op1=OP.add)
        M8 = cpool.tile([B, T], f32)  # 8*m  (osm carries the x16 h-scale)
        nc.vector.tensor_scalar(out=M8[:], in0=M[:], scalar1=8.0, scalar2=None,
                                op0=OP.mult)

        Sacc = cpool.tile([B, NPC * T], f32)
        LDacc = cpool.tile([B, NPC * T], f32)

        # ---- initial states ------------------------------------------------
        c_st = {}
        h_st = {}
        hT_st = {0: None, 1: None}
        for li in (0, 1):
            c_st[li] = state.tile([B, H], BF, name=f"c{li}", tag=f"c{li}")
            nc.vector.memset(c_st[li][:], 0.0)
            h_st[li] = state.tile([B, H], BF, name=f"h{li}", tag=f"h{li}")
            nc.vector.memset(h_st[li][:], 0.0)

        def k3(srcT):
            # [128, 512] fp8 T-layout -> [128, 4, 128] (k-chunk, batch)
            return srcT.rearrange("p (k b) -> p k b", k=KH)

        def alloc_gates():
            return [gpsum.tile([B, G // 2], f32, name="g", tag="g")
                    for _ in (0, 1)]

        def gates_part(halves, srcT, w_tile, part, start, stop):
            # fp8 DoubleRow: one instruction covers a k-pair (256 of K)
            k0 = 0 if part == "x" else KH
            s3 = k3(srcT[:])
            for half in (0, 1):
                gh = halves[half]
                for n in (0, 1):
                    osl = slice(512 * n, 512 * (n + 1))
                    wsl = slice(1024 * half + 512 * n,
                                1024 * half + 512 * (n + 1))
                    for j in (0, 1):
                        nc.tensor.matmul(
                            gh[:, osl], s3[:, 2 * j:2 * j + 2, :],
                            w_tile[:, k0 + 2 * j:k0 + 2 * j + 2, wsl],
                            start=(start and j == 0),
                            stop=(stop and j == 1),
                            perf_mode=DR)

        def cell(t, li, ghalves, make_hT):
            """bf16 LSTM cell. Gate banks: A=[f,i], B=[o,cg] (cg weights
            pre-doubled so one tanh scale serves all gates). h is carried
            16-scaled so the transposed copies cast straight to fp8."""
            gA, gB = ghalves
            mht = Mh[:, t:t + 1]
            m1t = M1[:, t:t + 1]
            m1ht = M1h[:, t:t + 1]
            m8t = M8[:, t:t + 1]

            # per-gate ACT granularity + 256-sliced c/tanh/o chain: the
            # shorter stages let ACT and DVE ping-pong across the two cell
            # chains instead of serializing on 1us full-width ops
            th = wp.tile([B, G], BF, name="th", tag="th")
            nc.scalar.activation(th[:, 0:512], gA[:, 0:512], AF.Tanh,
                                 scale=0.5 / 256.0)
            fp = wp.tile([B, H], BF, name="fp", tag="fp")
            nc.vector.tensor_scalar(out=fp[:], in0=th[:, 0:512], scalar1=mht,
                                    scalar2=m1ht, op0=OP.mult, op1=OP.add)
            nc.scalar.activation(th[:, 512:1024], gA[:, 512:1024], AF.Tanh,
                                 scale=0.5 / 256.0)
            ip = wp.tile([B, H], BF, name="ip", tag="ip")
            nc.vector.tensor_scalar(out=ip[:], in0=th[:, 512:1024], scalar1=mht,
                                    scalar2=mht, op0=OP.mult, op1=OP.add)
            nc.scalar.activation(th[:, 1536:2048], gB[:, 512:1024], AF.Tanh,
                                 scale=0.5 / 256.0)
            q = wp.tile([B, H], BF, name="q", tag="q")
            nc.vector.tensor_tensor(out=q[:], in0=ip[:], in1=th[:, 1536:2048],
                                    op=OP.mult)
            nc.scalar.activation(th[:, 1024:1536], gB[:, 0:512], AF.Tanh,
                                 scale=0.5 / 256.0)
            osm = wp.tile([B, H], BF, name="osm", tag="osm")
            nc.vector.tensor_scalar(out=osm[:], in0=th[:, 1024:1536], scalar1=m8t,
                                    scalar2=m8t, op0=OP.mult, op1=OP.add)

            c_new = state.tile([B, H], BF, name=f"c{li}", tag=f"c{li}")
            th_c = wp.tile([B, H], BF, name="th_c", tag="th_c")
            o16 = wp.tile([B, H], BF, name=f"o{li}", tag=f"o{li}")
            ps = tpsum.tile([128, H], BF, name="tp", tag="tp")
            for hf in (0, 1):
                sl = slice(256 * hf, 256 * (hf + 1))
                r_h = wp.tile([B, 256], BF, name="r_h", tag="r_h")
                nc.vector.tensor_tensor(out=r_h[:], in0=fp[:, sl],
                                        in1=c_st[li][:, sl], op=OP.mult)
                nc.vector.tensor_tensor(out=c_new[:, sl], in0=r_h[:],
                                        in1=q[:, sl], op=OP.add)
                nc.scalar.activation(th_c[:, sl], c_new[:, sl], AF.Tanh)
                nc.vector.tensor_tensor(out=o16[:, sl], in0=osm[:, sl],
                                        in1=th_c[:, sl], op=OP.mult)
                for kc in (2 * hf, 2 * hf + 1):
                    ksl = slice(128 * kc, 128 * (kc + 1))
                    nc.tensor.transpose(ps[:, ksl], o16[:, ksl], identb[:])
            c_st[li] = c_new
            h16 = state.tile([B, H], BF, name=f"h{li}", tag=f"h{li}")
            nc.vector.scalar_tensor_tensor(out=h16[:], in0=h_st[li][:],
                                           scalar=m1t, in1=o16[:],
                                           op0=OP.mult, op1=OP.add)
            h_st[li] = h16

            oT = wp.tile([128, H], FP8, name=f"oT{li}", tag=f"oT{li}")
            nc.scalar.copy(oT[:, 0:256], ps[:, 0:256])
            nc.vector.tensor_copy(out=oT[:, 256:512], in_=ps[:, 256:512])

            if make_hT:
                ps2 = tpsum.tile([128, H], BF, name="tp", tag="tp")
                for kc in range(KH):
                    sl = slice(128 * kc, 128 * (kc + 1))
                    nc.tensor.transpose(ps2[:, sl], h16[:, sl], identb[:])
                hT = state.tile([128, H], FP8, name=f"hT{li}", tag=f"hT{li}")
                nc.scalar.copy(hT[:], ps2[:])
                hT_st[li] = hT
            return oT

        def gather_x(t):
            xg = wp.tile([B, H], BF, name="xg", tag="xg")
            nc.gpsimd.indirect_dma_start(
                out=xg[:], out_offset=None, in_=ext["emb"][:, :],
                in_offset=IndirectOffsetOnAxis(ap=feat[:, t:t + 1], axis=0))
            ps = tpsum.tile([128, H], BF, name="tp", tag="tp")
            for kc in range(KH):
                sl = slice(128 * kc, 128 * (kc + 1))
                nc.tensor.transpose(ps[:, sl], xg[:, sl], identb[:])
            xT = wp.tile([128, H], FP8, name="xT", tag="xT")
            nc.vector.tensor_copy(out=xT[:], in_=ps[:])
            return xT

        def project(t, o1T):
            s3 = k3(o1T[:])
            for ci, (n0, n1) in enumerate(PCHUNKS):
                w = n1 - n0
                pp = ppsum.tile([128, 512], f32, name="pp", tag="pp")
                for j in (0, 1):
                    nc.tensor.matmul(pp[:, 0:w], s3[:, 2 * j:2 * j + 2, :],
                                     wsm[:, 2 * j:2 * j + 2, n0:n1],
                                     start=(j == 0), stop=(j == 1),
                                     perf_mode=DR)
                stt_scr = wp.tile([B, 512], f32, name="stt_scr", tag="stt_scr")
                nc.vector.scalar_tensor_tensor(
                    out=stt_scr[:, 0:w], in0=iota_v[:, n0:n1],
                    scalar=lab[:, t:t + 1], in1=pp[:, 0:w],
                    op0=OP.is_equal, op1=OP.mult,
                    accum_out=LDacc[:, NPC * t + ci:NPC * t + ci + 1])
                exp_scr = wp.tile([B, 512], f32, name="exp_scr", tag="exp_scr")
                nc.scalar.activation(
                    exp_scr[:, 0:w], pp[:, 0:w], AF.Exp, scale=1.0 / 256.0,
                    accum_out=Sacc[:, NPC * t + ci:NPC * t + ci + 1])

        # ---- software-pipelined main loop ---------------------------------
        xT_cur = gather_x(0)
        g0 = alloc_gates()
        gates_part(g0, xT_cur, w0, "x", start=True, stop=True)  # t=0: no rec
        o1T_prev = None
        for t in range(T):
            if t > 0:
                gates_part(g0, hT_st[0], w0, "h", start=False, stop=True)
            if t + 1 < T:
                xT_next = gather_x(t + 1)
            if o1T_prev is not None:
                project(t - 1, o1T_prev)
            g1 = None
            if t > 0:
                g1 = alloc_gates()
                gates_part(g1, hT_st[1], w1, "h", start=True, stop=False)
            o0T = cell(t, 0, g0, make_hT=(t + 1 < T))
            if g1 is None:
                g1 = alloc_gates()
                gates_part(g1, o0T, w1, "x", start=True, stop=True)
            else:
                gates_part(g1, o0T, w1, "x", start=False, stop=True)
            if t + 1 < T:
                g0 = alloc_gates()
                gates_part(g0, xT_next, w0, "x", start=True, stop=False)
                xT_cur = xT_next
            o1T = cell(t, 1, g1, make_hT=(t + 1 < T))
            o1T_prev = o1T
        project(T - 1, o1T_prev)

        nc.sync.dma_start(ext["S"][:, :], Sacc[:])
        nc.sync.dma_start(ext["LD"][:, :], LDacc[:])


def _build():
    if "nc" in _CACHE:
        return _CACHE["nc"]
    nc = bacc.Bacc("TRN2", target_bir_lowering=False, debug=False,
                   num_devices=NCORES)
    ext = {
        "features": nc.declare_dram_parameter("features", [B, T], dt.int32,
                                              isOutput=False),
        "labels_sh": nc.declare_dram_parameter("labels_sh", [B, T], dt.float32,
                                               isOutput=False),
        "seqlen": nc.declare_dram_parameter("seqlen", [B, 1], dt.float32,
                                            isOutput=False),
        "emb": nc.declare_dram_parameter("emb", [V, H], BF, isOutput=False),
        "w0": nc.declare_dram_parameter("w0", [2 * KH, 128, G], FP8,
                                        isOutput=False),
        "w1": nc.declare_dram_parameter("w1", [2 * KH, 128, G], FP8,
                                        isOutput=False),
        "wsm": nc.declare_dram_parameter("wsm", [KH, 128, VSH], FP8,
                                         isOutput=False),
        "S": nc.declare_dram_parameter("S", [B, NPC * T], dt.float32,
                                       isOutput=True),
        "LD": nc.declare_dram_parameter("LD", [B, NPC * T], dt.float32,
                                        isOutput=True),
    }
    with tile.TileContext(nc) as tc:
        _emit(nc, tc, ext)
    nc.compile()
    _CACHE["nc"] = nc
    return nc


def _pack_w(Wx, Wh):
    w = np.concatenate([np.asarray(Wx, np.float32), np.asarray(Wh, np.float32)],
                       axis=0)  # [2H, 4H] rows: x-part then h-part
    # reorder gate blocks [i, cg, f, o] -> [f, i, o, cg]
    w = np.concatenate([w[:, 1024:1536], w[:, 0:512], w[:, 1536:2048],
                        w[:, 512:1024]], axis=1)
    w = w * np.float32(FSCALE)
    w[:, 1536:2048] *= np.float32(2.0)  # cg: unified 0.5 tanh scale
    return np.ascontiguousarray(w.reshape(2 * KH, 128, G)).astype(NP_FP8)


def kernel(features, labels, seq_lengths, seq_mask, embedding,
           W0x, W0h, b0, W1x, W1h, b1, softmax_w, softmax_b,
           _trace_dir=None):
    for name, b in (("b0", b0), ("b1", b1), ("softmax_b", softmax_b)):
        if np.any(np.asarray(b, np.float32) != 0.0):
            raise NotImplementedError(f"{name} != 0 not supported")

    feats = np.ascontiguousarray(np.asarray(features, np.int32)[:, :T])
    labels_f = np.ascontiguousarray(
        np.asarray(labels, np.int32)[:, :T].astype(np.float32))
    slen = np.asarray(seq_lengths, np.int32).astype(np.float32).reshape(B, 1)
    mask = np.asarray(seq_mask, np.float32)[:, :T]
    emb = (np.asarray(embedding, np.float32) * np.float32(FSCALE)).astype(
        ml_dtypes.bfloat16)
    w0 = _pack_w(W0x, W0h)
    w1 = _pack_w(W1x, W1h)
    wsm_r = (np.asarray(softmax_w, np.float32) * np.float32(FSCALE)).reshape(
        KH, 128, V)

    nc = _build()
    in_maps = []
    for c in range(NCORES):
        in_maps.append({
            "features": feats,
            "labels_sh": labels_f - np.float32(c * VSH),
            "seqlen": slen,
            "emb": emb,
            "w0": w0,
            "w1": w1,
            "wsm": np.ascontiguousarray(
                wsm_r[:, :, c * VSH:(c + 1) * VSH]).astype(NP_FP8),
        })

    kwargs = {}
    if _trace_dir is not None:
        kwargs = dict(trace=True, tmpdir=_trace_dir)
    res = run_bass_kernel_spmd(nc, in_maps, list(range(NCORES)), **kwargs)
    _CACHE["last_results"] = res

    S = np.zeros((B, T), np.float64)
    LD = np.zeros((B, T), np.float64)
    for c in range(NCORES):
        S += np.asarray(res.results[c]["S"], np.float64).reshape(B, T, NPC).sum(-1)
        LD += np.asarray(res.results[c]["LD"], np.float64).reshape(B, T, NPC).sum(-1)
    LD /= 256.0

    xent = np.log(S) - LD
    loss_t = (xent * mask).sum(axis=0) / (mask.sum(axis=0) + 1e-12)
    cost = loss_t.mean()
    return np.asarray(cost, np.float32)


# revision 16
# speedup vs baseline: 1.3002x; 1.1519x over previous
"""Trainium2 Bass kernel: 2-layer LSTM language-model loss.

Reference: x = embedding[features]; 2-layer LSTM over T=64 steps with
sequence-length state freezing; logits = out @ softmax_w + softmax_b;
masked mean cross-entropy -> scalar.

Strategy (8 NeuronCores, SPMD, zero cross-core collectives):
  * Every core runs the identical full-batch (B=128) recurrence; the
    large projection is sharded over the vocab dim (1250 cols/core).
    Host gathers: xent = log(sum_c S_c) - sum_c LD_c.
  * All big matmuls run in fp8e4 with DoubleRow perf mode (2 k-chunks
    of 128 per instruction): weights, embedding and the h/o activations
    are scaled by 16 host/kernel-side so fp8 values stay normal; gate
    PSUM is 256x the true pre-activation and the tanh ACT reads with
    scale 0.5/256 (the cg gate's weight columns are pre-doubled so one
    scale serves all four gates -> one ACT per PSUM bank).
  * Cell elementwise chain is bf16 end-to-end (DVE 2x/4x modes); c and
    h are bf16 (h carried 16-scaled so transposed copies cast straight
    to fp8 without a multiply).
  * Masking is folded into the sigmoid affines as in the baseline:
    sig(x) = 0.5 tanh(0.5 x) + 0.5, fp = 0.5m th + (1 - 0.5m) etc.
  * S and LD are emitted per projection chunk ([B, 3T]) so the kernel
    skips the per-step add trees; the host sums the 3 chunks.

Assumes b0 = b1 = softmax_b = 0 (verified at runtime).
"""

import numpy as np
import ml_dtypes


def _ensure_path():
    try:
        import concourse  # noqa: F401
    except ImportError:
        import sys

        for p in ("/opt/trn_rl_repo", "/root/.axon_site/_ro/trn_rl_repo"):
            if p not in sys.path:
                sys.path.append(p)


_ensure_path()

from contextlib import ExitStack  # noqa: E402

import concourse.bass as bass  # noqa: E402
import concourse.bacc as bacc  # noqa: E402
import concourse.tile as tile  # noqa: E402
from concourse import mybir  # noqa: E402
from concourse.alu_op_type import AluOpType as OP  # noqa: E402
from concourse.bass import IndirectOffsetOnAxis  # noqa: E402
from concourse.bass_utils import run_bass_kernel_spmd  # noqa: E402
from concourse.masks import make_identity  # noqa: E402

dt = mybir.dt
AF = mybir.ActivationFunctionType
DR = mybir.MatmulPerfMode.DoubleRow

import os as _os

B = 128
T = int(_os.environ.get("KERNEL_T_OVERRIDE", "64"))
H = 512
V = 10000
NCORES = 8
VSH = V // NCORES  # 1250
G = 4 * H  # 2048
KH = H // 128  # 4 k-chunks per 512-wide contraction
FP8 = dt.float8e4
BF = dt.bfloat16
NP_FP8 = ml_dtypes.float8_e4m3
FSCALE = 16.0  # fp8 operand scale; products are 256x
# projection free-dim chunks (PSUM bank = 512 fp32)
PCHUNKS = [(0, 512), (512, 1024), (1024, VSH)]
NPC = len(PCHUNKS)

_CACHE: dict = {}


def _emit(nc, tc, ext):
    f32 = dt.float32
    with ExitStack() as ctx:
        cpool = ctx.enter_context(tc.tile_pool(name="const", bufs=1))
        state = ctx.enter_context(tc.tile_pool(name="state", bufs=2))
        wp = ctx.enter_context(tc.tile_pool(name="work", bufs=3))
        gpsum = ctx.enter_context(tc.tile_pool(name="gpsum", bufs=2, space="PSUM"))
        tpsum = ctx.enter_context(tc.tile_pool(name="tpsum", bufs=2, space="PSUM"))
        ppsum = ctx.enter_context(tc.tile_pool(name="ppsum", bufs=2, space="PSUM"))

        # ---- constants / inputs -------------------------------------------
        feat = cpool.tile([B, T], dt.int32)
        nc.sync.dma_start(feat[:], ext["features"][:, :])
        lab = cpool.tile([B, T], f32)
        nc.sync.dma_start(lab[:], ext["labels_sh"][:, :])
        slen = cpool.tile([B, 1], f32)
        nc.sync.dma_start(slen[:], ext["seqlen"][:, :])

        # per-k-chunk DMAs: first gate matmuls start before the full load
        w0 = cpool.tile([128, 2 * KH, G], FP8)
        for k in range(2 * KH):
            nc.sync.dma_start(w0[:, k, :], ext["w0"][k, :, :])
        w1 = cpool.tile([128, 2 * KH, G], FP8)
        for k in range(2 * KH):
            nc.sync.dma_start(w1[:, k, :], ext["w1"][k, :, :])
        wsm = cpool.tile([128, KH, VSH], FP8)
        nc.sync.dma_start(wsm[:], ext["wsm"][:, :, :].rearrange("k p n -> p k n"))

        identb = cpool.tile([128, 128], BF)
        make_identity(nc, identb[:])

        iota_v = cpool.tile([128, VSH], f32)
        nc.gpsimd.iota(iota_v[:], pattern=[[1, VSH]], base=0, channel_multiplier=0,
                       allow_small_or_imprecise_dtypes=True)
        iota_t = cpool.tile([128, T], f32)
        nc.gpsimd.iota(iota_t[:], pattern=[[1, T]], base=0, channel_multiplier=0,
                       allow_small_or_imprecise_dtypes=True)

        # masks: M = (t < seqlen), and derived per-step scalars
        M = cpool.tile([B, T], f32)
        nc.vector.tensor_scalar(out=M[:], in0=iota_t[:], scalar1=slen[:, 0:1],
                                scalar2=None, op0=OP.is_lt)
        Mh = cpool.tile([B, T], f32)  # 0.5*m
        nc.vector.tensor_scalar(out=Mh[:], in0=M[:], scalar1=0.5, scalar2=None,
                                op0=OP.mult)
        M1 = cpool.tile([B, T], f32)  # 1-m
        nc.vector.tensor_scalar(out=M1[:], in0=M[:], scalar1=-1.0, scalar2=1.0,
                                op0=OP.mult, op1=OP.add)
        M1h = cpool.tile([B, T], f32)  # 1-0.5*m
        nc.vector.tensor_scalar(out=M1h[:], in0=M[:], scalar1=-0.5, scalar2=1.0,
                                op0=OP.mult, op1=OP.add)
        M8 = cpool.tile([B, T], f32)  # 8*m  (osm carries the x16 h-scale)
        nc.vector.tensor_scalar(out=M8[:], in0=M[:], scalar1=8.0, scalar2=None,
                                op0=OP.mult)

        Sacc = cpool.tile([B, NPC * T], f32)
        LDacc = cpool.tile([B, NPC * T], f32)

        # ---- initial states ------------------------------------------------
        c_st = {}
        h_st = {}
        hT_st = {0: None, 1: None}
        for li in (0, 1):
            c_st[li] = state.tile([B, H], BF, name=f"c{li}", tag=f"c{li}")
            nc.vector.memset(c_st[li][:], 0.0)
            h_st[li] = state.tile([B, H], BF, name=f"h{li}", tag=f"h{li}")
            nc.vector.memset(h_st[li][:], 0.0)

        def k3(srcT):
            # [128, 512] fp8 T-layout -> [128, 4, 128] (k-chunk, batch)
            return srcT.rearrange("p (k b) -> p k b", k=KH)

        def alloc_gates():
            return [gpsum.tile([B, G // 2], f32, name="g", tag="g")
                    for _ in (0, 1)]

        def gates_part(halves, srcT, w_tile, part, start, stop):
            # fp8 DoubleRow: one instruction covers a k-pair (256 of K)
            k0 = 0 if part == "x" else KH
            s3 = k3(srcT[:])
            for half in (0, 1):
                gh = halves[half]
                for n in (0, 1):
                    osl = slice(512 * n, 512 * (n + 1))
                    wsl = slice(1024 * half + 512 * n,
                                1024 * half + 512 * (n + 1))
                    for j in (0, 1):
                        nc.tensor.matmul(
                            gh[:, osl], s3[:, 2 * j:2 * j + 2, :],
                            w_tile[:, k0 + 2 * j:k0 + 2 * j + 2, wsl],
                            start=(start and j == 0),
                            stop=(stop and j == 1),
                            perf_mode=DR)

        def cell(t, li, ghalves, make_hT):
            """bf16 LSTM cell. Gate banks: A=[f,i], B=[o,cg] (cg weights
            pre-doubled so one tanh scale serves all gates). h is carried
            16-scaled so the transposed copies cast straight to fp8."""
            gA, gB = ghalves
            mht = Mh[:, t:t + 1]
            m1t = M1[:, t:t + 1]
            m1ht = M1h[:, t:t + 1]
            m8t = M8[:, t:t + 1]

            # per-gate ACT granularity + 256-sliced c/tanh/o chain: the
            # shorter stages let ACT and DVE ping-pong across the two cell
            # chains instead of serializing on 1us full-width ops
            th = wp.tile([B, G], BF, name="th", tag="th")
            nc.scalar.activation(th[:, 0:512], gA[:, 0:512], AF.Tanh,
                                 scale=0.5 / 256.0)
            fp = wp.tile([B, H], BF, name="fp", tag="fp")
            nc.vector.tensor_scalar(out=fp[:], in0=th[:, 0:512], scalar1=mht,
                                    scalar2=m1ht, op0=OP.mult, op1=OP.add)
            nc.scalar.activation(th[:, 512:1024], gA[:, 512:1024], AF.Tanh,
                                 scale=0.5 / 256.0)
            ip = wp.tile([B, H], BF, name="ip", tag="ip")
            nc.vector.tensor_scalar(out=ip[:], in0=th[:, 512:1024], scalar1=mht,
                                    scalar2=mht, op0=OP.mult, op1=OP.add)
            nc.scalar.activation(th[:, 1536:2048], gB[:, 512:1024], AF.Tanh,
                                 scale=0.5 / 256.0)
            q = wp.tile([B, H], BF, name="q", tag="q")
            nc.vector.tensor_tensor(out=q[:], in0=ip[:], in1=th[:, 1536:2048],
                                    op=OP.mult)
            nc.scalar.activation(th[:, 1024:1536], gB[:, 0:512], AF.Tanh,
                                 scale=0.5 / 256.0)
            osm = wp.tile([B, H], BF, name="osm", tag="osm")
            nc.vector.tensor_scalar(out=osm[:], in0=th[:, 1024:1536], scalar1=m8t,
                                    scalar2=m8t, op0=OP.mult, op1=OP.add)

            c_new = state.tile([B, H], BF, name=f"c{li}", tag=f"c{li}")
            th_c = wp.tile([B, H], BF, name="th_c", tag="th_c")
            o16 = wp.tile([B, H], BF, name=f"o{li}", tag=f"o{li}")
            ps = tpsum.tile([128, H], BF, name="tp", tag="tp")
            for hf in (0, 1):
                sl = slice(256 * hf, 256 * (hf + 1))
                r_h = wp.tile([B, 256], BF, name="r_h", tag="r_h")
                nc.vector.tensor_tensor(out=r_h[:], in0=fp[:, sl],
                                        in1=c_st[li][:, sl], op=OP.mult)
                nc.vector.tensor_tensor(out=c_new[:, sl], in0=r_h[:],
                                        in1=q[:, sl], op=OP.add)
                nc.scalar.activation(th_c[:, sl], c_new[:, sl], AF.Tanh)
                nc.vector.tensor_tensor(out=o16[:, sl], in0=osm[:, sl],
                                        in1=th_c[:, sl], op=OP.mult)
                for kc in (2 * hf, 2 * hf + 1):
                    ksl = slice(128 * kc, 128 * (kc + 1))
                    nc.tensor.transpose(ps[:, ksl], o16[:, ksl], identb[:])
            c_st[li] = c_new
            h16 = state.tile([B, H], BF, name=f"h{li}", tag=f"h{li}")
            nc.vector.scalar_tensor_tensor(out=h16[:], in0=h_st[li][:],
                                           scalar=m1t, in1=o16[:],
                                           op0=OP.mult, op1=OP.add)
            h_st[li] = h16

            oT = wp.tile([128, H], FP8, name=f"oT{li}", tag=f"oT{li}")
            nc.scalar.copy(oT[:, 0:256], ps[:, 0:256])
            nc.vector.tensor_copy(out=oT[:, 256:512], in_=ps[:, 256:512])

            if make_hT:
                ps2 = tpsum.tile([128, H], BF, name="tp", tag="tp")
                for kc in range(KH):
                    sl = slice(128 * kc, 128 * (kc + 1))
                    nc.tensor.transpose(ps2[:, sl], h16[:, sl], identb[:])
                hT = state.tile([128, H], FP8, name=f"hT{li}", tag=f"hT{li}")
                nc.scalar.copy(hT[:], ps2[:])
                hT_st[li] = hT
            return oT

        def gather_xg(t):
            # layer0 x-part gates: one [B, 2048] bf16 row gather (256x scale)
            xg = wp.tile([B, G], BF, name="xg", tag="xg")
            nc.gpsimd.indirect_dma_start(
                out=xg[:], out_offset=None, in_=ext["xgtab"][:, :],
                in_offset=IndirectOffsetOnAxis(ap=feat[:, t:t + 1], axis=0))
            return xg

        def preload_xg(halves, xg, stop):
            # PSUM <- xg via identity matmuls (starts each slice's group)
            for half in (0, 1):
                gh = halves[half]
                for n in (0, 1):
                    osl = slice(512 * n, 512 * (n + 1))
                    xsl = slice(1024 * half + 512 * n,
                                1024 * half + 512 * (n + 1))
                    nc.tensor.matmul(gh[:, osl], identb[:], xg[:, xsl],
                                     start=True, stop=stop)

        def project(t, o1T):
            s3 = k3(o1T[:])
            for ci, (n0, n1) in enumerate(PCHUNKS):
                w = n1 - n0
                pp = ppsum.tile([128, 512], f32, name="pp", tag="pp")
                for j in (0, 1):
                    nc.tensor.matmul(pp[:, 0:w], s3[:, 2 * j:2 * j + 2, :],
                                     wsm[:, 2 * j:2 * j + 2, n0:n1],
                                     start=(j == 0), stop=(j == 1),
                                     perf_mode=DR)
                stt_scr = wp.tile([B, 512], f32, name="stt_scr", tag="stt_scr")
                nc.vector.scalar_tensor_tensor(
                    out=stt_scr[:, 0:w], in0=iota_v[:, n0:n1],
                    scalar=lab[:, t:t + 1], in1=pp[:, 0:w],
                    op0=OP.is_equal, op1=OP.mult,
                    accum_out=LDacc[:, NPC * t + ci:NPC * t + ci + 1])
                exp_scr = wp.tile([B, 512], f32, name="exp_scr", tag="exp_scr")
                nc.scalar.activation(
                    exp_scr[:, 0:w], pp[:, 0:w], AF.Exp, scale=1.0 / 256.0,
                    accum_out=Sacc[:, NPC * t + ci:NPC * t + ci + 1])

        # ---- software-pipelined main loop ---------------------------------
        xg_cur = gather_xg(0)
        g0 = alloc_gates()
        preload_xg(g0, xg_cur, stop=True)  # t=0: no recurrent part
        o1T_prev = None
        for t in range(T):
            if t > 0:
                gates_part(g0, hT_st[0], w0, "h", start=False, stop=True)
            if t + 1 < T:
                xg_next = gather_xg(t + 1)
            if o1T_prev is not None:
                project(t - 1, o1T_prev)
            g1 = None
            if t > 0:
                g1 = alloc_gates()
                gates_part(g1, hT_st[1], w1, "h", start=True, stop=False)
            o0T = cell(t, 0, g0, make_hT=(t + 1 < T))
            if g1 is None:
                g1 = alloc_gates()
                gates_part(g1, o0T, w1, "x", start=True, stop=True)
            else:
                gates_part(g1, o0T, w1, "x", start=False, stop=True)
            if t + 1 < T:
                g0 = alloc_gates()
                preload_xg(g0, xg_next, stop=False)
            o1T = cell(t, 1, g1, make_hT=(t + 1 < T))
            o1T_prev = o1T
        project(T - 1, o1T_prev)

        nc.sync.dma_start(ext["S"][:, :], Sacc[:])
        nc.sync.dma_start(ext["LD"][:, :], LDacc[:])


def _build():
    if "nc" in _CACHE:
        return _CACHE["nc"]
    nc = bacc.Bacc("TRN2", target_bir_lowering=False, debug=False,
                   num_devices=NCORES)
    ext = {
        "features": nc.declare_dram_parameter("features", [B, T], dt.int32,
                                              isOutput=False),
        "labels_sh": nc.declare_dram_parameter("labels_sh", [B, T], dt.float32,
                                               isOutput=False),
        "seqlen": nc.declare_dram_parameter("seqlen", [B, 1], dt.float32,
                                            isOutput=False),
        "emb": nc.declare_dram_parameter("emb", [V, H], BF, isOutput=False),
        "xgtab": nc.declare_dram_parameter("xgtab", [V, G], BF, isOutput=False),
        "w0": nc.declare_dram_parameter("w0", [2 * KH, 128, G], FP8,
                                        isOutput=False),
        "w1": nc.declare_dram_parameter("w1", [2 * KH, 128, G], FP8,
                                        isOutput=False),
        "wsm": nc.declare_dram_parameter("wsm", [KH, 128, VSH], FP8,
                                         isOutput=False),
        "S": nc.declare_dram_parameter("S", [B, NPC * T], dt.float32,
                                       isOutput=True),
        "LD": nc.declare_dram_parameter("LD", [B, NPC * T], dt.float32,
                                        isOutput=True),
    }
    with tile.TileContext(nc) as tc:
        _emit(nc, tc, ext)
    nc.compile()
    _CACHE["nc"] = nc
    return nc


def _pack_w(Wx, Wh):
    w = np.concatenate([np.asarray(Wx, np.float32), np.asarray(Wh, np.float32)],
                       axis=0)  # [2H, 4H] rows: x-part then h-part
    # reorder gate blocks [i, cg, f, o] -> [f, i, o, cg]
    w = np.concatenate([w[:, 1024:1536], w[:, 0:512], w[:, 1536:2048],
                        w[:, 512:1024]], axis=1)
    w = w * np.float32(FSCALE)
    w[:, 1536:2048] *= np.float32(2.0)  # cg: unified 0.5 tanh scale
    return np.ascontiguousarray(w.reshape(2 * KH, 128, G)).astype(NP_FP8)


def kernel(features, labels, seq_lengths, seq_mask, embedding,
           W0x, W0h, b0, W1x, W1h, b1, softmax_w, softmax_b,
           _trace_dir=None):
    for name, b in (("b0", b0), ("b1", b1), ("softmax_b", softmax_b)):
        if np.any(np.asarray(b, np.float32) != 0.0):
            raise NotImplementedError(f"{name} != 0 not supported")

    feats = np.ascontiguousarray(np.asarray(features, np.int32)[:, :T])
    labels_f = np.ascontiguousarray(
        np.asarray(labels, np.int32)[:, :T].astype(np.float32))
    slen = np.asarray(seq_lengths, np.int32).astype(np.float32).reshape(B, 1)
    mask = np.asarray(seq_mask, np.float32)[:, :T]
    emb = (np.asarray(embedding, np.float32) * np.float32(FSCALE)).astype(
        ml_dtypes.bfloat16)
    W0x32 = np.asarray(W0x, np.float32)
    W0x_r = np.concatenate([W0x32[:, 1024:1536], W0x32[:, 0:512],
                            W0x32[:, 1536:2048], 2.0 * W0x32[:, 512:1024]],
                           axis=1)
    xgtab = (np.asarray(embedding, np.float32) @ W0x_r
             * np.float32(FSCALE * FSCALE)).astype(ml_dtypes.bfloat16)
    w0 = _pack_w(W0x, W0h)
    w1 = _pack_w(W1x, W1h)
    wsm_r = (np.asarray(softmax_w, np.float32) * np.float32(FSCALE)).reshape(
        KH, 128, V)

    nc = _build()
    in_maps = []
    for c in range(NCORES):
        in_maps.append({
            "features": feats,
            "labels_sh": labels_f - np.float32(c * VSH),
            "seqlen": slen,
            "emb": emb,
            "xgtab": xgtab,
            "w0": w0,
            "w1": w1,
            "wsm": np.ascontiguousarray(
                wsm_r[:, :, c * VSH:(c + 1) * VSH]).astype(NP_FP8),
        })

    kwargs = {}
    if _trace_dir is not None:
        kwargs = dict(trace=True, tmpdir=_trace_dir)
    res = run_bass_kernel_spmd(nc, in_maps, list(range(NCORES)), **kwargs)
    _CACHE["last_results"] = res

    S = np.zeros((B, T), np.float64)
    LD = np.zeros((B, T), np.float64)
    for c in range(NCORES):
        S += np.asarray(res.results[c]["S"], np.float64).reshape(B, T, NPC).sum(-1)
        LD += np.asarray(res.results[c]["LD"], np.float64).reshape(B, T, NPC).sum(-1)
    LD /= 256.0

    xent = np.log(S) - LD
    loss_t = (xent * mask).sum(axis=0) / (mask.sum(axis=0) + 1e-12)
    cost = loss_t.mean()
    return np.asarray(cost, np.float32)
